# revision 1
# baseline (speedup 1.0000x reference)
"""GCN 2-layer classifier on 8 TRN2 NeuronCores.

Strategy (dst-sharded graph parallel, gather/scatter via GPSIMD + scan):
  - Nodes sharded 8 ways by id range (NSH=12544 logical rows per core, core 7
    zero-padded).  Each core computes hs1 = (x_shard @ W1) * dinv for its
    shard, transposes it to [16, NSH], and AllGathers across cores; the
    concatenated result IS the feature-major table layout
    table_T[(bank, feat), node_in_bank] = [128, NSH], DMA'd once into SBUF.
  - Edges sorted by dst on the host, bucketed per (core, src-bank,
    dst-range-chunk) into uniform-length int16 index streams (SPMD-identical
    structure, data differs per core).  Per chunk:
      * GPSIMD ap_gather pulls hs[src] along the free axis for all 8 banks in
        parallel (each Q7 core serves its bank's 16 feature partitions).
      * DVE tensor_tensor_scan computes a plain prefix sum over the
        dst-sorted message stream.
      * a second ap_gather extracts the prefix at per-dst segment boundaries;
        adjacent differences give per-(bank,dst) partial sums.
      * one PE matmul per 128 dsts contracts the partition axis against a
        block-identity selector, summing the 8 banks AND transposing to
        [dst, feat] in PSUM.
  - Symmetric normalization folds into the tables: out = dinv*(agg+hs[d]) + b
    with hs = h*dinv, so there is no per-edge norm work.
  - Layer 2 aggregates 16-dim features first (A@h commutes with @W2), then
    applies W2 + b2 and log-softmax on-chip.
"""

import sys

import numpy as np

sys.path.insert(0, "/opt/trn_rl_repo")

N_NODES = 100000
N_EDGES = 3200000
D_IN, D_HID, D_OUT = 128, 16, 2
NCORES = 8
P = 128
NSH = 12544          # shard rows per core (98 * 128)
TILES = NSH // P     # 98
NCHUNK = 14          # dst-range chunks per core
DCH = NSH // NCHUNK  # 896 dsts per chunk (= 7 node tiles)
TPC = DCH // P       # 7 tiles per chunk
NBANK = 8


def _host_prep(edge_index):
    """Sort edges by dst, bucket per (core, src-bank, dst-chunk), build
    uniform int16 gather/extraction index streams."""
    src = np.ascontiguousarray(edge_index[0]).astype(np.int64)
    dst = np.ascontiguousarray(edge_index[1]).astype(np.int64)

    deg = np.bincount(dst, minlength=N_NODES).astype(np.float64) + 1.0
    dinv = (1.0 / np.sqrt(deg)).astype(np.float32)

    order = np.argsort(dst, kind="stable")
    src_s = src[order]
    dst_s = dst[order]
    bank_s = src_s // NSH

    # cell id = ((core * NBANK) + bank) * NCHUNK + chunk, edges within a cell
    # stay dst-sorted under a stable sort by cell
    core_s = dst_s // NSH
    chunk_s = (dst_s % NSH) // DCH
    cell = (core_s * NBANK + bank_s) * NCHUNK + chunk_s
    cell_order = np.argsort(cell, kind="stable")
    src_c = src_s[cell_order]
    dst_c = dst_s[cell_order]
    cell_c = cell[cell_order]

    ncells = NCORES * NBANK * NCHUNK
    counts = np.bincount(cell_c, minlength=ncells)
    starts = np.zeros(ncells + 1, dtype=np.int64)
    np.cumsum(counts, out=starts[1:])

    # uniform padded stream length: slot 0 is a zero sentinel
    # round to multiples of 32 so every per-chunk int16 index slice starts
    # 4-byte aligned (GPSIMD reads indices in 32-bit words)
    nidx = int(counts.max()) + 1
    nidx = ((nidx + 31) // 32) * 32
    nx = DCH + 1
    nx = ((nx + 31) // 32) * 32

    gidx = np.zeros((NCORES, P, NCHUNK * (nidx // 16)), dtype=np.int16)
    xidx = np.zeros((NCORES, P, NCHUNK * (nx // 16)), dtype=np.int16)

    src_local = (src_c % NSH).astype(np.int32)
    rel_dst = (dst_c % NSH) % DCH

    for c in range(NCORES):
        for b in range(NBANK):
            rows = slice(b * 16, (b + 1) * 16)
            for k in range(NCHUNK):
                g = (c * NBANK + b) * NCHUNK + k
                a, e = starts[g], starts[g + 1]
                n = e - a
                # gather stream: [0] + bank-local src ids + pads(0)
                stream = np.zeros(nidx, dtype=np.int16)
                stream[1:1 + n] = src_local[a:e]
                gidx[c, rows, k * (nidx // 16):(k + 1) * (nidx // 16)] = (
                    stream.reshape(nidx // 16, 16).T
                )
                # extraction stream: prefix positions [0, cum(0), ..,
                # cum(DCH-1)] then pads repeating the last position
                cum = np.zeros(nx, dtype=np.int16)
                cnt = np.bincount(rel_dst[a:e], minlength=DCH)
                cum[1:DCH + 1] = np.cumsum(cnt)
                cum[DCH + 1:] = cum[DCH]
                xidx[c, rows, k * (nx // 16):(k + 1) * (nx // 16)] = (
                    cum.reshape(nx // 16, 16).T
                )

    return gidx, xidx, dinv, nidx, nx


def _build_program(nidx, nx, debug_taps=False):
    from contextlib import ExitStack

    import concourse.bass as bass
    import concourse.tile as tile
    from concourse import bacc, mybir
    from concourse.masks import make_identity

    f32 = mybir.dt.float32
    i16 = mybir.dt.int16

    nc = bacc.Bacc(
        "TRN2",
        target_bir_lowering=False,
        debug=False,
        enable_asserts=False,
        num_devices=NCORES,
    )

    # ---- kernel I/O ----
    NT = NCORES * TILES  # full-graph node tiles (layer-1 table is computed
    # locally on every core from the full x, replacing an AllGather)
    x_full = nc.dram_tensor("x_full", [NCORES * NSH, D_IN], f32, kind="ExternalInput")
    x_sh = nc.dram_tensor("x_shard", [NSH, D_IN], f32, kind="ExternalInput")
    dinvf_d = nc.dram_tensor("dinv_full", [NCORES * NSH], f32, kind="ExternalInput")
    w1_d = nc.dram_tensor("W1", [D_IN, D_HID], f32, kind="ExternalInput")
    b1_d = nc.dram_tensor("b1", [D_HID], f32, kind="ExternalInput")
    w2_d = nc.dram_tensor("W2", [D_HID, D_OUT], f32, kind="ExternalInput")
    b2_d = nc.dram_tensor("b2", [D_OUT], f32, kind="ExternalInput")
    dinv_d = nc.dram_tensor("dinv_shard", [NSH], f32, kind="ExternalInput")
    gidx_d = nc.dram_tensor("gidx", [P, NCHUNK * (nidx // 16)], i16, kind="ExternalInput")
    xidx_d = nc.dram_tensor("xidx", [P, NCHUNK * (nx // 16)], i16, kind="ExternalInput")
    sel_d = nc.dram_tensor("selmat", [P, D_HID], f32, kind="ExternalInput")
    out_d = nc.dram_tensor("out", [NSH, D_OUT], f32, kind="ExternalOutput")
    if debug_taps:
        dbg_tbl = nc.dram_tensor("dbg_tbl", [P, NSH], f32, kind="ExternalOutput")
        dbg_agg = nc.dram_tensor("dbg_agg", [NSH, D_HID], f32, kind="ExternalOutput")
        dbg_hs1 = nc.dram_tensor("dbg_hs1", [NSH, D_HID], f32, kind="ExternalOutput")
        dbg_msg = nc.dram_tensor("dbg_msg", [P, nidx], f32, kind="ExternalOutput")
        dbg_ex = nc.dram_tensor("dbg_ex", [P, nx], f32, kind="ExternalOutput")

    # internal DRAM: transposed shard bounce + transposed tables
    ag_in2 = nc.dram_tensor("ag_in2", [D_HID, NSH], f32)
    table1 = nc.dram_tensor("table1", [P, NSH], f32)
    table2 = nc.dram_tensor("table2", [P, NSH], f32, addr_space="Shared")

    groups = [list(range(NCORES))]

    with tile.TileContext(nc) as tc, ExitStack() as ctx:
        singles = ctx.enter_context(tc.tile_pool(name="singles", bufs=1))
        xpool = ctx.enter_context(tc.tile_pool(name="xload", bufs=3))
        xtp = ctx.enter_context(tc.tile_pool(name="xtsb", bufs=3))
        msgp = ctx.enter_context(tc.tile_pool(name="msg", bufs=2))
        scnp = ctx.enter_context(tc.tile_pool(name="scn", bufs=2))
        extp = ctx.enter_context(tc.tile_pool(name="ext", bufs=2))
        psA = ctx.enter_context(tc.tile_pool(name="psA", bufs=2, space="PSUM"))
        psB = ctx.enter_context(tc.tile_pool(name="psB", bufs=2, space="PSUM"))
        psW = ctx.enter_context(tc.tile_pool(name="psW", bufs=3, space="PSUM"))

        # ---- constants ----
        w1s = singles.tile([D_IN, D_HID], f32)
        nc.sync.dma_start(out=w1s[:], in_=w1_d[:, :])
        w2s = singles.tile([D_HID, D_OUT], f32)
        nc.sync.dma_start(out=w2s[:], in_=w2_d[:, :])
        b1s = singles.tile([P, D_HID], f32)
        nc.sync.dma_start(out=b1s[:], in_=b1_d.ap().unsqueeze(0).to_broadcast([P, D_HID]))
        b2s = singles.tile([P, D_OUT], f32)
        nc.sync.dma_start(out=b2s[:], in_=b2_d.ap().unsqueeze(0).to_broadcast([P, D_OUT]))
        sels = singles.tile([P, D_HID], f32)
        nc.sync.dma_start(out=sels[:], in_=sel_d[:, :])
        dinvs = singles.tile([P, TILES], f32)
        nc.sync.dma_start(out=dinvs[:], in_=bass.AP(dinv_d, 0, [[1, P], [P, TILES]]))
        dinvf = singles.tile([P, NT], f32)
        nc.sync.dma_start(out=dinvf[:], in_=bass.AP(dinvf_d, 0, [[1, P], [P, NT]]))
        ident = singles.tile([P, P], f32)
        make_identity(nc, ident[:])

        gidx = singles.tile([P, NCHUNK * (nidx // 16)], i16)
        nc.sync.dma_start(out=gidx[:], in_=gidx_d[:, :])
        xidx = singles.tile([P, NCHUNK * (nx // 16)], i16)
        nc.sync.dma_start(out=xidx[:], in_=xidx_d[:, :])

        hs1_loc = singles.tile([P, TILES, D_HID], f32)
        hs2_loc = singles.tile([P, TILES, D_HID], f32)
        agg1 = singles.tile([P, TILES, D_HID], f32)
        agg2 = singles.tile([P, TILES, D_HID], f32)
        tableT = singles.tile([P, NSH], f32)

        dinv_bc = dinvs[:].unsqueeze(2).to_broadcast([P, TILES, D_HID])

        # ---- phase A: hs1 = (x @ W1) * dinv; ship transposed shard ----
        def shard_to_table(hs_loc, ag_in, table):
            for t in range(TILES):
                tp = psA.tile([D_HID, P], f32, space="PSUM", tag="shT")
                nc.tensor.transpose(tp[:], hs_loc[:, t, :], ident[:])
                st = xtp.tile([D_HID, P], f32, tag="shstage")
                nc.vector.tensor_copy(st[:], tp[:])
                nc.sync.dma_start(
                    out=bass.AP(ag_in, t * P, [[NSH, D_HID], [1, P]]),
                    in_=st[:],
                )
            nc.gpsimd.collective_compute(
                "AllGather", mybir.AluOpType.bypass, replica_groups=groups,
                ins=[ag_in.ap().opt()], outs=[table.ap().opt()],
            )
            nc.sync.dma_start(out=tableT[:], in_=table[:, :])

        for t in range(TILES):
            xt = xpool.tile([P, D_IN], f32)
            nc.sync.dma_start(out=xt[:], in_=x_sh[t * P:(t + 1) * P, :])
            xt_ps = psA.tile([P, P], f32, space="PSUM", tag="shT")
            nc.tensor.transpose(xt_ps[:], xt[:], ident[:])
            xT = xtp.tile([P, P], f32)
            nc.vector.tensor_copy(xT[:], xt_ps[:])
            h_ps = psB.tile([P, D_HID], f32, space="PSUM", tag="small")
            nc.tensor.matmul(out=h_ps[:], lhsT=xT[:], rhs=w1s[:], start=True, stop=True)
            nc.vector.tensor_scalar_mul(hs1_loc[:, t, :], h_ps[:], dinvs[:, t:t + 1])

        # full layer-1 table computed locally on every core (no collective):
        # per 128-node tile: hs_T = W1^T @ (dinv*x)^T -> [16, 128], staged in
        # 7-tile groups and written to this core's private table1 DRAM.
        GRP = 7
        for b in range(NBANK):
            for g in range(TILES // GRP):
                stg = xtp.tile([D_HID, GRP * P], f32, tag="stg")
                for j in range(GRP):
                    t = b * TILES + g * GRP + j
                    xt = xpool.tile([P, D_IN], f32, tag="xf")
                    nc.sync.dma_start(out=xt[:], in_=x_full[t * P:(t + 1) * P, :])
                    xs = xpool.tile([P, D_IN], f32, tag="xs")
                    nc.vector.tensor_scalar_mul(xs[:], xt[:], dinvf[:, t:t + 1])
                    xs_ps = psA.tile([P, P], f32, space="PSUM", tag="shT")
                    nc.tensor.transpose(xs_ps[:], xs[:], ident[:])
                    xsT = xtp.tile([P, P], f32, tag="xT2")
                    nc.vector.tensor_copy(xsT[:], xs_ps[:])
                    h_ps = psB.tile([D_HID, P], f32, space="PSUM", tag="small")
                    nc.tensor.matmul(out=h_ps[:], lhsT=w1s[:], rhs=xsT[:],
                                     start=True, stop=True)
                    nc.vector.tensor_copy(stg[:, j * P:(j + 1) * P], h_ps[:])
                nc.sync.dma_start(
                    out=bass.AP(table1, (16 * b) * NSH + g * GRP * P,
                                [[NSH, D_HID], [1, GRP * P]]),
                    in_=stg[:],
                )
        nc.sync.dma_start(out=tableT[:], in_=table1[:, :])

        if debug_taps:
            nc.sync.dma_start(out=dbg_tbl[:, :], in_=tableT[:])
            shp = bass.AP(dbg_hs1, 0, [[D_HID, P], [P * D_HID, TILES], [1, D_HID]])
            nc.sync.dma_start(out=shp, in_=hs1_loc[:])

        # ---- edge aggregation ----
        def aggregate(aggbuf, tap=False):
            for k in range(NCHUNK):
                msg = msgp.tile([P, nidx], f32, tag="msg")
                nc.gpsimd.ap_gather(
                    out_ap=msg[:], in_ap=tableT[:],
                    idxs_ap=gidx[:, k * (nidx // 16):(k + 1) * (nidx // 16)],
                    channels=P, num_elems=NSH, d=1, num_idxs=nidx,
                )
                nc.vector.memset(msg[:, 0:1], 0.0)
                scn = scnp.tile([P, nidx], f32, tag="scn")
                nc.vector.tensor_tensor_scan(
                    out=scn[:], data0=msg[:], data1=msg[:], initial=0.0,
                    op0=mybir.AluOpType.add, op1=mybir.AluOpType.bypass,
                )
                ex = extp.tile([P, nx], f32, tag="ex")
                nc.gpsimd.ap_gather(
                    out_ap=ex[:], in_ap=scn[:],
                    idxs_ap=xidx[:, k * (nx // 16):(k + 1) * (nx // 16)],
                    channels=P, num_elems=nidx, d=1, num_idxs=nx,
                )
                dif = extp.tile([P, DCH], f32, tag="dif")
                nc.vector.tensor_sub(dif[:], ex[:, 1:DCH + 1], ex[:, 0:DCH])
                if tap and k == 0:
                    nc.sync.dma_start(out=dbg_msg[:, :], in_=msg[:])
                    nc.sync.dma_start(out=dbg_ex[:, :], in_=ex[:])
                for j in range(TPC):
                    ps = psW.tile([P, D_HID], f32, space="PSUM")
                    nc.tensor.matmul(
                        out=ps[:], lhsT=dif[:, j * P:(j + 1) * P], rhs=sels[:],
                        start=True, stop=True,
                    )
                    nc.vector.tensor_copy(aggbuf[:, k * TPC + j, :], ps[:])

        aggregate(agg1, tap=debug_taps)
        if debug_taps:
            sap = bass.AP(dbg_agg, 0, [[D_HID, P], [P * D_HID, TILES], [1, D_HID]])
            nc.sync.dma_start(out=sap, in_=agg1[:])

        # ---- layer-1 epilogue ----
        t1 = singles.tile([P, TILES, D_HID], f32)
        nc.vector.tensor_add(out=t1[:], in0=agg1[:], in1=hs1_loc[:])
        nc.vector.tensor_mul(out=t1[:], in0=t1[:], in1=dinv_bc)
        b1_bc = b1s[:].unsqueeze(1).to_broadcast([P, TILES, D_HID])
        nc.vector.tensor_add(out=t1[:], in0=t1[:], in1=b1_bc)
        nc.scalar.activation(out=t1[:], in_=t1[:], func=mybir.ActivationFunctionType.Relu)
        nc.vector.tensor_mul(out=hs2_loc[:], in0=t1[:], in1=dinv_bc)

        shard_to_table(hs2_loc, ag_in2, table2)

        aggregate(agg2)

        # ---- layer-2 epilogue: y = (dinv*(agg2+hs2)) @ W2 + b2; log_softmax
        t2 = singles.tile([P, TILES, D_HID], f32)
        nc.vector.tensor_add(out=t2[:], in0=agg2[:], in1=hs2_loc[:])
        nc.vector.tensor_mul(out=t2[:], in0=t2[:], in1=dinv_bc)

        fin = singles.tile([P, TILES, D_OUT], f32)
        for t in range(TILES):
            tp_ps = psA.tile([D_HID, P], f32, space="PSUM", tag="shT")
            nc.tensor.transpose(tp_ps[:], t2[:, t, :], ident[:])
            t2T = xtp.tile([D_HID, P], f32, tag="t2T")
            nc.vector.tensor_copy(t2T[:], tp_ps[:])
            y_ps = psB.tile([P, D_OUT], f32, space="PSUM", tag="small")
            nc.tensor.matmul(out=y_ps[:], lhsT=t2T[:], rhs=w2s[:], start=True, stop=True)
            nc.vector.tensor_add(out=fin[:, t, :], in0=y_ps[:], in1=b2s[:])

        # log-softmax over 2 classes, batched over [P, TILES]
        mx = singles.tile([P, TILES], f32)
        nc.vector.tensor_max(out=mx[:], in0=fin[:, :, 0], in1=fin[:, :, 1])
        mx_bc = mx[:].unsqueeze(2).to_broadcast([P, TILES, D_OUT])
        zc = singles.tile([P, TILES, D_OUT], f32)
        nc.vector.tensor_sub(out=zc[:], in0=fin[:], in1=mx_bc)
        ez = singles.tile([P, TILES, D_OUT], f32)
        nc.scalar.activation(out=ez[:], in_=zc[:], func=mybir.ActivationFunctionType.Exp)
        sm = singles.tile([P, TILES], f32)
        nc.vector.tensor_add(out=sm[:], in0=ez[:, :, 0], in1=ez[:, :, 1])
        ls = singles.tile([P, TILES], f32)
        nc.scalar.activation(out=ls[:], in_=sm[:], func=mybir.ActivationFunctionType.Ln)
        ls_bc = ls[:].unsqueeze(2).to_broadcast([P, TILES, D_OUT])
        res = singles.tile([P, TILES, D_OUT], f32)
        nc.vector.tensor_sub(out=res[:], in0=zc[:], in1=ls_bc)

        out_ap = bass.AP(out_d, 0, [[D_OUT, P], [P * D_OUT, TILES], [1, D_OUT]])
        nc.sync.dma_start(out=out_ap, in_=res[:])

    nc.compile()
    return nc


def _build_noop():
    """Tiny program for calibrating the PJRT/axon transport overhead."""
    from contextlib import ExitStack

    import concourse.tile as tile
    from concourse import bacc, mybir

    f32 = mybir.dt.float32
    nc = bacc.Bacc(
        "TRN2", target_bir_lowering=False, debug=False,
        enable_asserts=False, num_devices=NCORES,
    )
    z_in = nc.dram_tensor("z_in", [P, P], f32, kind="ExternalInput")
    z_out = nc.dram_tensor("z_out", [P, P], f32, kind="ExternalOutput")
    with tile.TileContext(nc) as tc, ExitStack() as ctx:
        sb = ctx.enter_context(tc.tile_pool(name="sb", bufs=1))
        t = sb.tile([P, P], f32)
        nc.sync.dma_start(out=t[:], in_=z_in[:, :])
        nc.sync.dma_start(out=z_out[:, :], in_=t[:])
    nc.compile()
    return nc


_CACHE = {}


def _make_in_maps(inputs_np, gidx, xidx, dinv):
    x = np.asarray(inputs_np["x"], dtype=np.float32)
    x_pad = np.zeros((NCORES * NSH, D_IN), dtype=np.float32)
    x_pad[:N_NODES] = x
    dinv_pad = np.ones(NCORES * NSH, dtype=np.float32)
    dinv_pad[:N_NODES] = dinv
    selmat = np.tile(np.eye(D_HID, dtype=np.float32), (NBANK, 1))

    in_maps = []
    for c in range(NCORES):
        in_maps.append({
            "x_full": x_pad,
            "dinv_full": dinv_pad,
            "x_shard": np.ascontiguousarray(x_pad[c * NSH:(c + 1) * NSH]),
            "W1": np.asarray(inputs_np["W1"], dtype=np.float32),
            "b1": np.asarray(inputs_np["b1"], dtype=np.float32),
            "W2": np.asarray(inputs_np["W2"], dtype=np.float32),
            "b2": np.asarray(inputs_np["b2"], dtype=np.float32),
            "dinv_shard": np.ascontiguousarray(dinv_pad[c * NSH:(c + 1) * NSH]),
            "gidx": np.ascontiguousarray(gidx[c]),
            "xidx": np.ascontiguousarray(xidx[c]),
            "selmat": selmat,
        })
    return in_maps


def kernel(x, W1, b1, W2, b2, edge_index):
    from concourse.bass_utils import run_bass_kernel_spmd

    inputs_np = {"x": x, "W1": W1, "b1": b1, "W2": W2, "b2": b2}
    edge_index = np.asarray(edge_index)

    gidx, xidx, dinv, nidx, nx = _host_prep(edge_index)

    key = (nidx, nx)
    if key not in _CACHE:
        _CACHE[key] = _build_program(nidx, nx)
    nc = _CACHE[key]

    in_maps = _make_in_maps(inputs_np, gidx, xidx, dinv)

    res = run_bass_kernel_spmd(nc, in_maps, core_ids=list(range(NCORES)))
    shards = [res.results[c]["out"] for c in range(NCORES)]
    out = np.concatenate(shards, axis=0)[:N_NODES]
    return np.ascontiguousarray(out.astype(np.float32))



# revision 2
# speedup vs baseline: 21.1252x; 21.1252x over previous
"""GCN 2-layer classifier on 8 TRN2 NeuronCores.

Strategy (dst-sharded graph parallel, gather/scatter via GPSIMD + scan):
  - Nodes sharded 8 ways by id range (NSH=12544 logical rows per core, core 7
    zero-padded).  Each core receives ONLY its own x shard (bf16 to halve
    transfer), computes hs1 = (x_shard @ W1) * dinv, transposes it to
    [16, NSH], and AllGathers across cores; the concatenated result IS the
    feature-major table layout table_T[(bank, feat), node_in_bank] =
    [128, NSH], DMA'd once into SBUF.  Same for layer 2.
  - Edges sorted by dst on the host, bucketed per (core, src-bank,
    dst-range-chunk) into uniform-length int16 index streams (SPMD-identical
    structure, data differs per core).  Per chunk:
      * GPSIMD ap_gather pulls hs[src] along the free axis for all 8 banks in
        parallel (each Q7 core serves its bank's 16 feature partitions).
      * DVE tensor_tensor_scan computes a plain prefix sum over the
        dst-sorted message stream.
      * a second ap_gather extracts the prefix at per-dst segment boundaries;
        adjacent differences give per-(bank,dst) partial sums.
      * one PE matmul per 128 dsts contracts the partition axis against a
        block-identity selector, summing the 8 banks AND transposing to
        [dst, feat] in PSUM.
  - Symmetric normalization folds into the tables: out = dinv*(agg+hs[d]) + b
    with hs = h*dinv, so there is no per-edge norm work.
  - Layer 2 aggregates 16-dim features first (A@h commutes with @W2), then
    applies W2 + b2 and log-softmax on-chip.
"""

import sys

import numpy as np

sys.path.insert(0, "/opt/trn_rl_repo")

N_NODES = 100000
N_EDGES = 3200000
D_IN, D_HID, D_OUT = 128, 16, 2
NCORES = 8
P = 128
NSH = 12544          # shard rows per core (98 * 128)
TILES = NSH // P     # 98
NCHUNK = 14          # dst-range chunks per core
DCH = NSH // NCHUNK  # 896 dsts per chunk (= 7 node tiles)
TPC = DCH // P       # 7 tiles per chunk
NBANK = 8


def _host_prep(edge_index):
    """Sort edges by dst, bucket per (core, src-bank, dst-chunk), build
    uniform int16 gather/extraction index streams."""
    src = np.ascontiguousarray(edge_index[0]).astype(np.int64)
    dst = np.ascontiguousarray(edge_index[1]).astype(np.int64)

    deg = np.bincount(dst, minlength=N_NODES).astype(np.float64) + 1.0
    dinv = (1.0 / np.sqrt(deg)).astype(np.float32)

    order = np.argsort(dst, kind="stable")
    src_s = src[order]
    dst_s = dst[order]
    bank_s = src_s // NSH

    # cell id = ((core * NBANK) + bank) * NCHUNK + chunk, edges within a cell
    # stay dst-sorted under a stable sort by cell
    core_s = dst_s // NSH
    chunk_s = (dst_s % NSH) // DCH
    cell = (core_s * NBANK + bank_s) * NCHUNK + chunk_s
    cell_order = np.argsort(cell, kind="stable")
    src_c = src_s[cell_order]
    dst_c = dst_s[cell_order]
    cell_c = cell[cell_order]

    ncells = NCORES * NBANK * NCHUNK
    counts = np.bincount(cell_c, minlength=ncells)
    starts = np.zeros(ncells + 1, dtype=np.int64)
    np.cumsum(counts, out=starts[1:])

    # uniform padded stream length: slot 0 is a zero sentinel
    # round to multiples of 32 so every per-chunk int16 index slice starts
    # 4-byte aligned (GPSIMD reads indices in 32-bit words)
    nidx = int(counts.max()) + 1
    nidx = ((nidx + 31) // 32) * 32
    nx = DCH + 1
    nx = ((nx + 31) // 32) * 32

    gidx = np.zeros((NCORES, P, NCHUNK * (nidx // 16)), dtype=np.int16)
    xidx = np.zeros((NCORES, P, NCHUNK * (nx // 16)), dtype=np.int16)

    src_local = (src_c % NSH).astype(np.int32)
    rel_dst = (dst_c % NSH) % DCH

    for c in range(NCORES):
        for b in range(NBANK):
            rows = slice(b * 16, (b + 1) * 16)
            for k in range(NCHUNK):
                g = (c * NBANK + b) * NCHUNK + k
                a, e = starts[g], starts[g + 1]
                n = e - a
                # gather stream: [0] + bank-local src ids + pads(0)
                stream = np.zeros(nidx, dtype=np.int16)
                stream[1:1 + n] = src_local[a:e]
                gidx[c, rows, k * (nidx // 16):(k + 1) * (nidx // 16)] = (
                    stream.reshape(nidx // 16, 16).T
                )
                # extraction stream: prefix positions [0, cum(0), ..,
                # cum(DCH-1)] then pads repeating the last position
                cum = np.zeros(nx, dtype=np.int16)
                cnt = np.bincount(rel_dst[a:e], minlength=DCH)
                cum[1:DCH + 1] = np.cumsum(cnt)
                cum[DCH + 1:] = cum[DCH]
                xidx[c, rows, k * (nx // 16):(k + 1) * (nx // 16)] = (
                    cum.reshape(nx // 16, 16).T
                )

    return gidx, xidx, dinv, nidx, nx


def _build_program(nidx, nx):
    from contextlib import ExitStack

    import concourse.bass as bass
    import concourse.tile as tile
    from concourse import bacc, mybir
    from concourse.masks import make_identity

    f32 = mybir.dt.float32
    bf16 = mybir.dt.bfloat16
    i16 = mybir.dt.int16

    nc = bacc.Bacc(
        "TRN2",
        target_bir_lowering=False,
        debug=False,
        enable_asserts=False,
        num_devices=NCORES,
    )

    # ---- kernel I/O ----
    x_sh = nc.dram_tensor("x_shard", [NSH, D_IN], bf16, kind="ExternalInput")
    w1_d = nc.dram_tensor("W1", [D_IN, D_HID], f32, kind="ExternalInput")
    b1_d = nc.dram_tensor("b1", [D_HID], f32, kind="ExternalInput")
    w2_d = nc.dram_tensor("W2", [D_HID, D_OUT], f32, kind="ExternalInput")
    b2_d = nc.dram_tensor("b2", [D_OUT], f32, kind="ExternalInput")
    dinv_d = nc.dram_tensor("dinv_shard", [NSH], f32, kind="ExternalInput")
    gidx_d = nc.dram_tensor("gidx", [P, NCHUNK * (nidx // 16)], i16, kind="ExternalInput")
    xidx_d = nc.dram_tensor("xidx", [P, NCHUNK * (nx // 16)], i16, kind="ExternalInput")
    sel_d = nc.dram_tensor("selmat", [P, D_HID], f32, kind="ExternalInput")
    out_d = nc.dram_tensor("out", [NSH, D_OUT], f32, kind="ExternalOutput")

    # internal DRAM: transposed shard bounces + transposed (gathered) tables
    ag_in1 = nc.dram_tensor("ag_in1", [D_HID, NSH], f32)
    ag_in2 = nc.dram_tensor("ag_in2", [D_HID, NSH], f32)
    table1 = nc.dram_tensor("table1", [P, NSH], f32, addr_space="Shared")
    table2 = nc.dram_tensor("table2", [P, NSH], f32, addr_space="Shared")

    groups = [list(range(NCORES))]

    with tile.TileContext(nc) as tc, ExitStack() as ctx:
        singles = ctx.enter_context(tc.tile_pool(name="singles", bufs=1))
        xpool = ctx.enter_context(tc.tile_pool(name="xload", bufs=3))
        xtp = ctx.enter_context(tc.tile_pool(name="xtsb", bufs=3))
        msgp = ctx.enter_context(tc.tile_pool(name="msg", bufs=2))
        scnp = ctx.enter_context(tc.tile_pool(name="scn", bufs=2))
        extp = ctx.enter_context(tc.tile_pool(name="ext", bufs=2))
        psA = ctx.enter_context(tc.tile_pool(name="psA", bufs=2, space="PSUM"))
        psB = ctx.enter_context(tc.tile_pool(name="psB", bufs=2, space="PSUM"))
        psW = ctx.enter_context(tc.tile_pool(name="psW", bufs=3, space="PSUM"))

        # ---- constants ----
        w1s = singles.tile([D_IN, D_HID], f32)
        nc.sync.dma_start(out=w1s[:], in_=w1_d[:, :])
        w2s = singles.tile([D_HID, D_OUT], f32)
        nc.sync.dma_start(out=w2s[:], in_=w2_d[:, :])
        b1s = singles.tile([P, D_HID], f32)
        nc.sync.dma_start(out=b1s[:], in_=b1_d.ap().unsqueeze(0).to_broadcast([P, D_HID]))
        b2s = singles.tile([P, D_OUT], f32)
        nc.sync.dma_start(out=b2s[:], in_=b2_d.ap().unsqueeze(0).to_broadcast([P, D_OUT]))
        sels = singles.tile([P, D_HID], f32)
        nc.sync.dma_start(out=sels[:], in_=sel_d[:, :])
        dinvs = singles.tile([P, TILES], f32)
        nc.sync.dma_start(out=dinvs[:], in_=bass.AP(dinv_d, 0, [[1, P], [P, TILES]]))
        ident = singles.tile([P, P], f32)
        make_identity(nc, ident[:])

        gidx = singles.tile([P, NCHUNK * (nidx // 16)], i16)
        nc.sync.dma_start(out=gidx[:], in_=gidx_d[:, :])
        xidx = singles.tile([P, NCHUNK * (nx // 16)], i16)
        nc.sync.dma_start(out=xidx[:], in_=xidx_d[:, :])

        hs1_loc = singles.tile([P, TILES, D_HID], f32)
        hs2_loc = singles.tile([P, TILES, D_HID], f32)
        agg1 = singles.tile([P, TILES, D_HID], f32)
        agg2 = singles.tile([P, TILES, D_HID], f32)
        tableT = singles.tile([P, NSH], f32)

        dinv_bc = dinvs[:].unsqueeze(2).to_broadcast([P, TILES, D_HID])

        # ---- shard -> feature-major full-graph table via AllGather ----
        def shard_to_table(hs_loc, ag_in, table):
            GRP = 7
            for g in range(TILES // GRP):
                stg = xtp.tile([D_HID, GRP * P], f32, tag="stg")
                for j in range(GRP):
                    t = g * GRP + j
                    tp = psA.tile([D_HID, P], f32, space="PSUM", tag="shT")
                    nc.tensor.transpose(tp[:], hs_loc[:, t, :], ident[:])
                    nc.vector.tensor_copy(stg[:, j * P:(j + 1) * P], tp[:])
                nc.sync.dma_start(
                    out=bass.AP(ag_in, g * GRP * P, [[NSH, D_HID], [1, GRP * P]]),
                    in_=stg[:],
                )
            nc.gpsimd.collective_compute(
                "AllGather", mybir.AluOpType.bypass, replica_groups=groups,
                ins=[ag_in.ap().opt()], outs=[table.ap().opt()],
            )
            nc.sync.dma_start(out=tableT[:], in_=table[:, :])

        # ---- phase A: hs1 = (x_shard @ W1) * dinv (node-major) ----
        for t in range(TILES):
            xt = xpool.tile([P, D_IN], bf16)
            nc.sync.dma_start(out=xt[:], in_=x_sh[t * P:(t + 1) * P, :])
            xtf = xpool.tile([P, D_IN], f32, tag="xf32")
            nc.vector.tensor_copy(xtf[:], xt[:])
            xt_ps = psA.tile([P, P], f32, space="PSUM", tag="shT")
            nc.tensor.transpose(xt_ps[:], xtf[:], ident[:])
            xT = xtp.tile([P, P], f32)
            nc.vector.tensor_copy(xT[:], xt_ps[:])
            h_ps = psB.tile([P, D_HID], f32, space="PSUM", tag="small")
            nc.tensor.matmul(out=h_ps[:], lhsT=xT[:], rhs=w1s[:], start=True, stop=True)
            nc.vector.tensor_scalar_mul(hs1_loc[:, t, :], h_ps[:], dinvs[:, t:t + 1])

        shard_to_table(hs1_loc, ag_in1, table1)

        # ---- edge aggregation ----
        def aggregate(aggbuf):
            for k in range(NCHUNK):
                msg = msgp.tile([P, nidx], f32, tag="msg")
                nc.gpsimd.ap_gather(
                    out_ap=msg[:], in_ap=tableT[:],
                    idxs_ap=gidx[:, k * (nidx // 16):(k + 1) * (nidx // 16)],
                    channels=P, num_elems=NSH, d=1, num_idxs=nidx,
                )
                nc.vector.memset(msg[:, 0:1], 0.0)
                scn = scnp.tile([P, nidx], f32, tag="scn")
                nc.vector.tensor_tensor_scan(
                    out=scn[:], data0=msg[:], data1=msg[:], initial=0.0,
                    op0=mybir.AluOpType.add, op1=mybir.AluOpType.bypass,
                )
                ex = extp.tile([P, nx], f32, tag="ex")
                nc.gpsimd.ap_gather(
                    out_ap=ex[:], in_ap=scn[:],
                    idxs_ap=xidx[:, k * (nx // 16):(k + 1) * (nx // 16)],
                    channels=P, num_elems=nidx, d=1, num_idxs=nx,
                )
                dif = extp.tile([P, DCH], f32, tag="dif")
                nc.vector.tensor_sub(dif[:], ex[:, 1:DCH + 1], ex[:, 0:DCH])
                for j in range(TPC):
                    ps = psW.tile([P, D_HID], f32, space="PSUM")
                    nc.tensor.matmul(
                        out=ps[:], lhsT=dif[:, j * P:(j + 1) * P], rhs=sels[:],
                        start=True, stop=True,
                    )
                    nc.vector.tensor_copy(aggbuf[:, k * TPC + j, :], ps[:])

        aggregate(agg1)

        # ---- layer-1 epilogue ----
        t1 = singles.tile([P, TILES, D_HID], f32)
        nc.vector.tensor_add(out=t1[:], in0=agg1[:], in1=hs1_loc[:])
        nc.vector.tensor_mul(out=t1[:], in0=t1[:], in1=dinv_bc)
        b1_bc = b1s[:].unsqueeze(1).to_broadcast([P, TILES, D_HID])
        nc.vector.tensor_add(out=t1[:], in0=t1[:], in1=b1_bc)
        nc.scalar.activation(out=t1[:], in_=t1[:], func=mybir.ActivationFunctionType.Relu)
        nc.vector.tensor_mul(out=hs2_loc[:], in0=t1[:], in1=dinv_bc)

        shard_to_table(hs2_loc, ag_in2, table2)

        aggregate(agg2)

        # ---- layer-2 epilogue: y = (dinv*(agg2+hs2)) @ W2 + b2; log_softmax
        t2 = singles.tile([P, TILES, D_HID], f32)
        nc.vector.tensor_add(out=t2[:], in0=agg2[:], in1=hs2_loc[:])
        nc.vector.tensor_mul(out=t2[:], in0=t2[:], in1=dinv_bc)

        fin = singles.tile([P, TILES, D_OUT], f32)
        for t in range(TILES):
            tp_ps = psA.tile([D_HID, P], f32, space="PSUM", tag="shT")
            nc.tensor.transpose(tp_ps[:], t2[:, t, :], ident[:])
            t2T = xtp.tile([D_HID, P], f32, tag="t2T")
            nc.vector.tensor_copy(t2T[:], tp_ps[:])
            y_ps = psB.tile([P, D_OUT], f32, space="PSUM", tag="small")
            nc.tensor.matmul(out=y_ps[:], lhsT=t2T[:], rhs=w2s[:], start=True, stop=True)
            nc.vector.tensor_add(out=fin[:, t, :], in0=y_ps[:], in1=b2s[:])

        # log-softmax over 2 classes, batched over [P, TILES]
        mx = singles.tile([P, TILES], f32)
        nc.vector.tensor_max(out=mx[:], in0=fin[:, :, 0], in1=fin[:, :, 1])
        mx_bc = mx[:].unsqueeze(2).to_broadcast([P, TILES, D_OUT])
        zc = singles.tile([P, TILES, D_OUT], f32)
        nc.vector.tensor_sub(out=zc[:], in0=fin[:], in1=mx_bc)
        ez = singles.tile([P, TILES, D_OUT], f32)
        nc.scalar.activation(out=ez[:], in_=zc[:], func=mybir.ActivationFunctionType.Exp)
        sm = singles.tile([P, TILES], f32)
        nc.vector.tensor_add(out=sm[:], in0=ez[:, :, 0], in1=ez[:, :, 1])
        ls = singles.tile([P, TILES], f32)
        nc.scalar.activation(out=ls[:], in_=sm[:], func=mybir.ActivationFunctionType.Ln)
        ls_bc = ls[:].unsqueeze(2).to_broadcast([P, TILES, D_OUT])
        res = singles.tile([P, TILES, D_OUT], f32)
        nc.vector.tensor_sub(out=res[:], in0=zc[:], in1=ls_bc)

        out_ap = bass.AP(out_d, 0, [[D_OUT, P], [P * D_OUT, TILES], [1, D_OUT]])
        nc.sync.dma_start(out=out_ap, in_=res[:])

    nc.compile()
    return nc


def _build_noop():
    """Tiny program for calibrating the PJRT/axon transport overhead."""
    from contextlib import ExitStack

    import concourse.tile as tile
    from concourse import bacc, mybir

    f32 = mybir.dt.float32
    nc = bacc.Bacc(
        "TRN2", target_bir_lowering=False, debug=False,
        enable_asserts=False, num_devices=NCORES,
    )
    z_in = nc.dram_tensor("z_in", [P, P], f32, kind="ExternalInput")
    z_out = nc.dram_tensor("z_out", [P, P], f32, kind="ExternalOutput")
    with tile.TileContext(nc) as tc, ExitStack() as ctx:
        sb = ctx.enter_context(tc.tile_pool(name="sb", bufs=1))
        t = sb.tile([P, P], f32)
        nc.sync.dma_start(out=t[:], in_=z_in[:, :])
        nc.sync.dma_start(out=z_out[:, :], in_=t[:])
    nc.compile()
    return nc


_CACHE = {}


def _make_in_maps(inputs_np, gidx, xidx, dinv):
    import ml_dtypes

    x = np.asarray(inputs_np["x"], dtype=np.float32)
    x_pad = np.zeros((NCORES * NSH, D_IN), dtype=np.float32)
    x_pad[:N_NODES] = x
    x_bf = x_pad.astype(ml_dtypes.bfloat16)
    dinv_pad = np.ones(NCORES * NSH, dtype=np.float32)
    dinv_pad[:N_NODES] = dinv
    selmat = np.tile(np.eye(D_HID, dtype=np.float32), (NBANK, 1))

    in_maps = []
    for c in range(NCORES):
        in_maps.append({
            "x_shard": np.ascontiguousarray(x_bf[c * NSH:(c + 1) * NSH]),
            "W1": np.asarray(inputs_np["W1"], dtype=np.float32),
            "b1": np.asarray(inputs_np["b1"], dtype=np.float32),
            "W2": np.asarray(inputs_np["W2"], dtype=np.float32),
            "b2": np.asarray(inputs_np["b2"], dtype=np.float32),
            "dinv_shard": np.ascontiguousarray(dinv_pad[c * NSH:(c + 1) * NSH]),
            "gidx": np.ascontiguousarray(gidx[c]),
            "xidx": np.ascontiguousarray(xidx[c]),
            "selmat": selmat,
        })
    return in_maps


def kernel(x, W1, b1, W2, b2, edge_index):
    from concourse.bass_utils import run_bass_kernel_spmd

    inputs_np = {"x": x, "W1": W1, "b1": b1, "W2": W2, "b2": b2}
    edge_index = np.asarray(edge_index)

    gidx, xidx, dinv, nidx, nx = _host_prep(edge_index)

    key = (nidx, nx)
    if key not in _CACHE:
        _CACHE[key] = _build_program(nidx, nx)
    nc = _CACHE[key]

    in_maps = _make_in_maps(inputs_np, gidx, xidx, dinv)

    res = run_bass_kernel_spmd(nc, in_maps, core_ids=list(range(NCORES)))
    shards = [res.results[c]["out"] for c in range(NCORES)]
    out = np.concatenate(shards, axis=0)[:N_NODES]
    return np.ascontiguousarray(out.astype(np.float32))
